# revision 1
# baseline (speedup 1.0000x reference)
"""BitLinear (ternary-weight linear with int8 activation quantization) on 8 trn2 cores.

y = (clip(round(x/x_scale),-128,127) * x_scale) @ (clip(round(w/w_scale),-1,1) * w_scale).T
  x_scale = max(max|x|, eps)/127   (per-tensor)
  w_scale = max(mean|w|, eps)      (per-tensor)

Sharding: tensor-parallel over out_features (11008 = 8 x 1376), x replicated.
Launch A computes per-core partial reductions (max|x| shard, sum|w| shard);
host combines 16 scalars; launch B does quantize + exact-integer bf16 matmul.
"""

import numpy as np
from contextlib import ExitStack

import concourse.bass as bass
import concourse.tile as tile
from concourse import bacc, mybir
from concourse.bass_utils import run_bass_kernel_spmd

# problem shapes (hardcoded per contract)
B, T, I, O = 4, 2048, 4096, 11008
TOK = B * T                  # 8192
N_CORES = 8
O_SH = O // N_CORES          # 1376
TOK_SH = TOK // N_CORES      # 1024
EPS = 1e-5
MAGIC = 12582912.0           # 1.5 * 2**23: fp32 add forces round-to-nearest-even int
F32 = mybir.dt.float32
BF16 = mybir.dt.bfloat16

# launch B tiling
TB = 256                     # tokens per streaming block (2 PSUM m-tiles)
NBLK = TOK // TB             # 32
KT = I // 128                # 32 k-tiles
CH = 8                       # k-tiles per x DMA chunk (CH*TB*4B*128 = 1MB)
NCH = KT // CH               # 4 chunks per block
WCH = 2                      # k-tiles per w prologue chunk
OB = (512, 512, 352)         # out-feature split per PSUM bank (sum = 1376)
OB_OFF = (0, 512, 1024)
EARLY = 4                    # blocks run slice-0-only while w slices 1/2 load


def _build_reduce():
    nc = bacc.Bacc("TRN2", target_bir_lowering=False, debug=False,
                   num_devices=N_CORES)
    # shards reshaped host-side to [128, *] row-major views
    xs = nc.dram_tensor("xs", [128, TOK_SH * I // 128], F32, kind="ExternalInput").ap()
    ws = nc.dram_tensor("ws", [128, O_SH * I // 128], F32, kind="ExternalInput").ap()
    partials = nc.dram_tensor("partials", [1, 2], F32, kind="ExternalOutput").ap()

    NX = 16
    FX = xs.shape[1] // NX    # 2048
    NW = 16
    FW = ws.shape[1] // NW    # 2752

    with tile.TileContext(nc) as tc:
        with ExitStack() as ctx:
            io = ctx.enter_context(tc.tile_pool(name="io", bufs=4))
            stats = ctx.enter_context(tc.tile_pool(name="stats", bufs=1))
            xstat = stats.tile([128, NX], F32)
            wstat = stats.tile([128, NW], F32)
            for i in range(NX):
                t = io.tile([128, FX], F32, tag="xin")
                nc.sync.dma_start(t[:], xs[:, i * FX:(i + 1) * FX])
                nc.vector.tensor_reduce(xstat[:, i:i + 1], t[:],
                                        axis=mybir.AxisListType.X,
                                        op=mybir.AluOpType.max,
                                        apply_absolute_value=True)
            for i in range(NW):
                t = io.tile([128, FW], F32, tag="win")
                nc.sync.dma_start(t[:], ws[:, i * FW:(i + 1) * FW])
                nc.vector.tensor_reduce(wstat[:, i:i + 1], t[:],
                                        axis=mybir.AxisListType.X,
                                        op=mybir.AluOpType.add,
                                        apply_absolute_value=True)
            xr = stats.tile([128, 1], F32)
            wr = stats.tile([128, 1], F32)
            nc.vector.tensor_reduce(xr[:], xstat[:], axis=mybir.AxisListType.X,
                                    op=mybir.AluOpType.max)
            nc.vector.tensor_reduce(wr[:], wstat[:], axis=mybir.AxisListType.X,
                                    op=mybir.AluOpType.add)
            # reduce across partitions on host is avoided: do it on device
            from concourse import bass_isa
            xrr = stats.tile([128, 1], F32)
            wrr = stats.tile([128, 1], F32)
            nc.gpsimd.partition_all_reduce(xrr[:], xr[:], channels=128,
                                           reduce_op=bass_isa.ReduceOp.max)
            nc.gpsimd.partition_all_reduce(wrr[:], wr[:], channels=128,
                                           reduce_op=bass_isa.ReduceOp.add)
            nc.sync.dma_start(partials[0:1, 0:1], xrr[0:1, :])
            nc.sync.dma_start(partials[0:1, 1:2], wrr[0:1, :])
    nc.compile()
    return nc


def _build_matmul():
    nc = bacc.Bacc("TRN2", target_bir_lowering=False, debug=False,
                   num_devices=N_CORES)
    xT = nc.dram_tensor("xT", [I, TOK], F32, kind="ExternalInput").ap()
    wT = nc.dram_tensor("wT", [I, O_SH], F32, kind="ExternalInput").ap()
    consts = nc.dram_tensor("consts", [1, 8], F32, kind="ExternalInput").ap()
    out = nc.dram_tensor("out", [TOK, O_SH], F32, kind="ExternalOutput").ap()

    xTr = xT.rearrange("(kt p) t -> p kt t", p=128)   # [128, KT, TOK]
    wTr = wT.rearrange("(kt p) o -> p kt o", p=128)   # [128, KT, O_SH]

    with tile.TileContext(nc) as tc:
        with ExitStack() as ctx:
            const_pool = ctx.enter_context(tc.tile_pool(name="const", bufs=1))
            wq_pool = ctx.enter_context(tc.tile_pool(name="wq", bufs=1))
            stage = ctx.enter_context(tc.tile_pool(name="stage", bufs=2))
            rnd = ctx.enter_context(tc.tile_pool(name="rnd", bufs=2))
            wstage = ctx.enter_context(tc.tile_pool(name="wstage", bufs=2))
            wrnd = ctx.enter_context(tc.tile_pool(name="wrnd", bufs=2))
            xq_pool = ctx.enter_context(tc.tile_pool(name="xq", bufs=4))
            out_pool = ctx.enter_context(tc.tile_pool(name="out", bufs=4))
            psum = ctx.enter_context(tc.tile_pool(name="psum", bufs=6, space="PSUM"))

            sb_c = const_pool.tile([128, 8], F32)
            nc.sync.dma_start(sb_c[:], consts.to_broadcast((128, 8)))
            inv_w = sb_c[:, 0:1]
            inv_x = sb_c[:, 1:2]
            out_scale = sb_c[:, 2:3]

            # SBUF-resident ternarized weight shard, bf16 [128, KT, O_SH]
            wq = wq_pool.tile([128, KT, O_SH], BF16)

            def quant_w_slice(b):
                o0, ow = OB_OFF[b], OB[b]
                for c in range(KT // WCH):
                    wf = wstage.tile([128, WCH, ow], F32, tag="wstage",
                                     name=f"wf{b}_{c}")
                    nc.sync.dma_start(wf[:], wTr[:, c * WCH:(c + 1) * WCH,
                                              o0:o0 + ow])
                    wr_ = wrnd.tile([128, WCH, ow], F32, tag="wrnd",
                                    name=f"wr{b}_{c}")
                    # round(w * inv_w) in magic space (ACT: out = in*scale + bias)
                    nc.scalar.activation(wr_[:], wf[:],
                                         mybir.ActivationFunctionType.Copy,
                                         bias=MAGIC, scale=inv_w)
                    # clip to [-1, 1] in magic space, subtract magic, cast bf16
                    nc.vector.tensor_scalar(wr_[:], wr_[:], MAGIC + 1.0, MAGIC - 1.0,
                                            op0=mybir.AluOpType.min,
                                            op1=mybir.AluOpType.max)
                    nc.vector.tensor_scalar(
                        wq[:, c * WCH:(c + 1) * WCH, o0:o0 + ow],
                        wr_[:], -MAGIC, None, op0=mybir.AluOpType.add)

            xq_tiles = {}

            def quant_x_block(tb):
                t0 = tb * TB
                xq = xq_pool.tile([128, KT, TB], BF16, tag="xq", name=f"xq{tb}")
                xq_tiles[tb] = xq
                for c in range(NCH):
                    xf = stage.tile([128, CH, TB], F32, tag="stage",
                                    name=f"xf{tb}_{c}")
                    nc.sync.dma_start(xf[:], xTr[:, c * CH:(c + 1) * CH,
                                              t0:t0 + TB])
                    xr_ = rnd.tile([128, CH, TB], F32, tag="rnd",
                                   name=f"xr{tb}_{c}")
                    nc.scalar.activation(xr_[:], xf[:],
                                         mybir.ActivationFunctionType.Copy,
                                         bias=MAGIC, scale=inv_x)
                    # no clip needed: |x|/x_scale <= 127 by construction
                    nc.vector.tensor_scalar(
                        xq[:, c * CH:(c + 1) * CH, :],
                        xr_[:], -MAGIC, None, op0=mybir.AluOpType.add)

            def mm_j(tb, j, bs):
                """matmul groups for m-tile j of block tb, psum banks bs,
                drain + store joint [128, O_SH] when bs covers all slices."""
                xq = xq_tiles[tb]
                ps = {}
                for b in bs:
                    ps[b] = psum.tile([128, 512], F32, tag="ps",
                                      name=f"ps{tb}_{j}_{b}")
                    for k in range(KT):
                        nc.tensor.matmul(ps[b][:, :OB[b]],
                                         xq[:, k, j * 128:(j + 1) * 128],
                                         wq[:, k, OB_OFF[b]:OB_OFF[b] + OB[b]],
                                         start=(k == 0), stop=(k == KT - 1))
                t0 = tb * TB + j * 128
                for b in bs:
                    ob = out_pool.tile([128, 512], F32, tag="ob",
                                       name=f"ob{tb}_{j}_{b}")
                    nc.scalar.mul(ob[:, :OB[b]], ps[b][:, :OB[b]], out_scale)
                    nc.sync.dma_start(
                        out[t0:t0 + 128, OB_OFF[b]:OB_OFF[b] + OB[b]],
                        ob[:, :OB[b]])

            # emission order tuned so the DMA queue feeds PE without stalls:
            # w slice 0 + first x blocks, then remaining w slices interleaved;
            # the first EARLY blocks run slice 0 only while slices 1/2 load.
            quant_w_slice(0)
            quant_x_block(0)
            quant_x_block(1)
            quant_x_block(2)
            quant_w_slice(1)
            quant_x_block(3)
            quant_w_slice(2)
            for b in range(3):
                for tb in range(EARLY):
                    for j in range(TB // 128):
                        mm_j(tb, j, [b])
            for tb in range(EARLY, NBLK):
                quant_x_block(tb)
                for j in range(TB // 128):
                    mm_j(tb, j, [0, 1, 2])
    nc.compile()
    return nc


_cache = {}


def _get_ncs():
    if "A" not in _cache:
        _cache["A"] = _build_reduce()
        _cache["B"] = _build_matmul()
    return _cache["A"], _cache["B"]


def _run(nc, in_maps, core_ids):
    try:
        return run_bass_kernel_spmd(nc, in_maps, core_ids)
    except Exception:
        import time as _t
        _t.sleep(10)  # transient tunnel/device hiccups recover on retry
        return run_bass_kernel_spmd(nc, in_maps, core_ids)


def kernel(x: np.ndarray, weight: np.ndarray) -> np.ndarray:
    ncA, ncB = _get_ncs()
    core_ids = list(range(N_CORES))

    x = np.asarray(x)
    weight = np.asarray(weight)
    assert x.shape == (B, T, I) and weight.shape == (O, I), (x.shape, weight.shape)
    x_flat = np.ascontiguousarray(x.reshape(TOK, I), dtype=np.float32)
    weight = np.ascontiguousarray(weight, dtype=np.float32)

    # ---- launch A: partial reductions over disjoint shards ----
    in_A = [{
        "xs": x_flat[i * TOK_SH:(i + 1) * TOK_SH].reshape(128, TOK_SH * I // 128),
        "ws": weight[i * O_SH:(i + 1) * O_SH].reshape(128, O_SH * I // 128),
    } for i in range(N_CORES)]
    resA = _run(ncA, in_A, core_ids)
    parts = np.stack([resA.results[i]["partials"][0] for i in range(N_CORES)])
    absmax = np.float32(parts[:, 0].max())
    wmean = np.float32(np.float32(parts[:, 1].sum(dtype=np.float64)) /
                       np.float32(O * I))
    x_scale = np.float32(max(absmax, np.float32(EPS))) / np.float32(127.0)
    w_scale = np.float32(max(wmean, np.float32(EPS)))
    consts = np.zeros((1, 8), dtype=np.float32)
    consts[0, 0] = np.float32(1.0) / w_scale
    consts[0, 1] = np.float32(1.0) / x_scale
    consts[0, 2] = x_scale * w_scale

    # ---- launch B: quantized matmul, tensor-parallel over out_features ----
    xT = np.ascontiguousarray(x_flat.T)               # [I, TOK]
    wTf = weight.T                                    # [I, O] view
    in_B = [{
        "xT": xT,
        "wT": np.ascontiguousarray(wTf[:, i * O_SH:(i + 1) * O_SH]),
        "consts": consts,
    } for i in range(N_CORES)]
    resB = _run(ncB, in_B, core_ids)
    out = np.concatenate([resB.results[i]["out"] for i in range(N_CORES)], axis=1)
    return out.reshape(B, T, O)



# revision 3
# speedup vs baseline: 1.0950x; 1.0950x over previous
"""BitLinear (ternary-weight linear with int8 activation quantization) on 8 trn2 cores.

y = (clip(round(x/x_scale),-128,127) * x_scale) @ (clip(round(w/w_scale),-1,1) * w_scale).T
  x_scale = max(max|x|, eps)/127   (per-tensor)
  w_scale = max(mean|w|, eps)      (per-tensor)

Sharding: tensor-parallel over out_features (11008 = 8 x 1376), x replicated.
Single device launch per core: quantize (magic-number rounding) + exact-integer
bf16 matmul. Per-tensor scales are two scalar reductions; they are computed
host-side and passed in as constants, so the launch is pure streaming compute.

The emission plan software-pipelines the startup: w slice 0 + x block 0 are
interleaved k-ordered so the PE starts within ~10us, early blocks run on the
slices already resident while the remaining w slices stream in, then catch-up
passes (which need no new x DMA) fill the PE while x prefetch rebuilds.
"""

import numpy as np
from contextlib import ExitStack

import concourse.bass as bass
import concourse.tile as tile
from concourse import bacc, mybir
from concourse.bass_utils import run_bass_kernel_spmd

# problem shapes (hardcoded per contract)
B, T, I, O = 4, 2048, 4096, 11008
TOK = B * T                  # 8192
N_CORES = 8
O_SH = O // N_CORES          # 1376
EPS = 1e-5
MAGIC = 12582912.0           # 1.5 * 2**23: fp32 add forces round-to-nearest-even int
F32 = mybir.dt.float32
BF16 = mybir.dt.bfloat16

# tiling
TB = 256                     # tokens per streaming block
NBLK = TOK // TB             # 32
KT = I // 128                # 32 k-tiles
CH = 4                       # k-tiles per x DMA chunk (CH*TB*4B*128 = 512KB)
NCH = KT // CH               # 8 chunks per block
WCH = 2                      # k-tiles per w chunk
NWCH = KT // WCH             # 16 chunks per slice
OB = (512, 512, 352)         # out-feature split per PSUM bank (sum = 1376)
OB_OFF = (0, 512, 1024)


def _interleave(a, b, na, nb):
    """merge two op lists, taking na from a then nb from b, repeating."""
    out, ia, ib = [], 0, 0
    while ia < len(a) or ib < len(b):
        out.extend(a[ia:ia + na]); ia += na
        out.extend(b[ib:ib + nb]); ib += nb
    return out


def _make_plan():
    """Emission plan: list of ('w', s, c) / ('x', tb, c) / ('mmb', tb, banks)."""
    ops = []
    W = lambda s: [("w", s, c) for c in range(NWCH)]
    X = lambda tb: [("x", tb, c) for c in range(NCH)]
    # startup: x block0 and w slice0 interleaved, k-ordered on both sides
    ops += [("x", 0, 0), ("x", 0, 1)]
    ops += _interleave(W(0), X(0)[2:], 3, 1)
    ops += [("mmb", 0, (0,))]
    ops += X(1)
    ops += [("mmb", 1, (0,))]
    # slice1 streams while blocks 2,3 load and run on slice0
    ops += _interleave(W(1), X(2) + X(3), 2, 1)
    ops += [("mmb", 2, (0,))]
    ops += [("mmb", 0, (1,)), ("mmb", 1, (1,))]      # catch-up: banked xq, no DMA
    ops += [("mmb", 3, (0,))]
    # slice2 streams while blocks 4,5 load; catch-up work keeps PE busy
    ops += _interleave(W(2), X(4) + X(5), 2, 1)
    ops += [("mmb", 2, (1,)), ("mmb", 3, (1,))]
    ops += [("mmb", 0, (2,)), ("mmb", 1, (2,))]
    ops += [("mmb", 2, (2,)), ("mmb", 3, (2,))]
    # steady state with 2-block x prefetch
    for tb in range(4, NBLK):
        if tb + 2 < NBLK:
            ops += X(tb + 2)
        ops += [("mmb", tb, (0, 1, 2))]
    return ops


def _build_matmul(plan=None):
    nc = bacc.Bacc("TRN2", target_bir_lowering=False, debug=False,
                   num_devices=N_CORES)
    xT = nc.dram_tensor("xT", [I, TOK], F32, kind="ExternalInput").ap()
    wT = nc.dram_tensor("wT", [I, O_SH], F32, kind="ExternalInput").ap()
    consts = nc.dram_tensor("consts", [1, 8], F32, kind="ExternalInput").ap()
    out = nc.dram_tensor("out", [TOK, O_SH], F32, kind="ExternalOutput").ap()

    xTr = xT.rearrange("(kt p) t -> p kt t", p=128)   # [128, KT, TOK]
    wTr = wT.rearrange("(kt p) o -> p kt o", p=128)   # [128, KT, O_SH]

    if plan is None:
        plan = _make_plan()

    with tile.TileContext(nc) as tc:
        with ExitStack() as ctx:
            const_pool = ctx.enter_context(tc.tile_pool(name="const", bufs=1))
            wq_pool = ctx.enter_context(tc.tile_pool(name="wq", bufs=1))
            stage = ctx.enter_context(tc.tile_pool(name="stage", bufs=2))
            wstage = ctx.enter_context(tc.tile_pool(name="wstage", bufs=2))
            xq_pool = ctx.enter_context(tc.tile_pool(name="xq", bufs=6))
            out_pool = ctx.enter_context(tc.tile_pool(name="out", bufs=4))
            psum = ctx.enter_context(tc.tile_pool(name="psum", bufs=6, space="PSUM"))

            sb_c = const_pool.tile([128, 8], F32)
            nc.sync.dma_start(sb_c[:], consts.to_broadcast((128, 8)))
            inv_w = sb_c[:, 0:1]
            inv_x = sb_c[:, 1:2]
            out_scale = sb_c[:, 2:3]

            # SBUF-resident ternarized weight shard, bf16 [128, KT, O_SH]
            wq = wq_pool.tile([128, KT, O_SH], BF16)

            def quant_w_chunk(s, c):
                o0, ow = OB_OFF[s], OB[s]
                k0 = c * WCH
                wf = wstage.tile([128, WCH, ow], F32, tag="wstage",
                                 name=f"wf{s}_{c}")
                nc.sync.dma_start(wf[:], wTr[:, k0:k0 + WCH, o0:o0 + ow])
                # round(w * inv_w) in magic space (ACT: out = in*scale + bias)
                nc.scalar.activation(wf[:], wf[:],
                                     mybir.ActivationFunctionType.Copy,
                                     bias=MAGIC, scale=inv_w)
                # clip to [-1, 1] in magic space
                nc.vector.tensor_scalar(wf[:], wf[:], MAGIC + 1.0, MAGIC - 1.0,
                                        op0=mybir.AluOpType.min,
                                        op1=mybir.AluOpType.max)
                # subtract magic, cast bf16 into resident wq
                nc.vector.tensor_scalar(
                    wq[:, k0:k0 + WCH, o0:o0 + ow],
                    wf[:], -MAGIC, None, op0=mybir.AluOpType.add)

            xq_tiles = {}

            def quant_x_chunk(tb, c):
                t0 = tb * TB
                if tb not in xq_tiles:
                    xq_tiles[tb] = xq_pool.tile([128, KT, TB], BF16, tag="xq",
                                                name=f"xq{tb}")
                xq = xq_tiles[tb]
                k0 = c * CH
                xf = stage.tile([128, CH, TB], F32, tag="stage",
                                name=f"xf{tb}_{c}")
                nc.sync.dma_start(xf[:], xTr[:, k0:k0 + CH, t0:t0 + TB])
                nc.scalar.activation(xf[:], xf[:],
                                     mybir.ActivationFunctionType.Copy,
                                     bias=MAGIC, scale=inv_x)
                # no clip needed: |x|/x_scale <= 127 by construction
                nc.vector.tensor_scalar(
                    xq[:, k0:k0 + CH, :],
                    xf[:], -MAGIC, None, op0=mybir.AluOpType.add)

            def mm_block(tb, banks):
                """k-outer/bank-inner matmuls for both j-tiles of block tb."""
                xq = xq_tiles[tb]
                t0 = tb * TB
                for j in range(TB // 128):
                    ps = {}
                    for b in banks:
                        ps[b] = psum.tile([128, 512], F32, tag="ps",
                                          name=f"ps{tb}_{j}_{b}")
                    for k in range(KT):
                        for b in banks:
                            nc.tensor.matmul(ps[b][:, :OB[b]],
                                             xq[:, k, j * 128:(j + 1) * 128],
                                             wq[:, k, OB_OFF[b]:OB_OFF[b] + OB[b]],
                                             start=(k == 0), stop=(k == KT - 1))
                    for b in banks:
                        ob = out_pool.tile([128, 512], F32, tag="ob",
                                           name=f"ob{tb}_{j}_{b}")
                        nc.scalar.mul(ob[:, :OB[b]], ps[b][:, :OB[b]], out_scale)
                        nc.sync.dma_start(
                            out[t0 + j * 128:t0 + j * 128 + 128,
                                OB_OFF[b]:OB_OFF[b] + OB[b]],
                            ob[:, :OB[b]])

            for op in plan:
                if op[0] == "w":
                    quant_w_chunk(op[1], op[2])
                elif op[0] == "x":
                    quant_x_chunk(op[1], op[2])
                else:
                    mm_block(op[1], op[2])
    nc.compile()
    return nc


_cache = {}


def _get_nc():
    if "B" not in _cache:
        _cache["B"] = _build_matmul()
    return _cache["B"]


def _run(nc, in_maps, core_ids):
    try:
        return run_bass_kernel_spmd(nc, in_maps, core_ids)
    except Exception:
        import time as _t
        _t.sleep(10)  # transient tunnel/device hiccups recover on retry
        return run_bass_kernel_spmd(nc, in_maps, core_ids)


def kernel(x: np.ndarray, weight: np.ndarray) -> np.ndarray:
    ncB = _get_nc()
    core_ids = list(range(N_CORES))

    x = np.asarray(x)
    weight = np.asarray(weight)
    assert x.shape == (B, T, I) and weight.shape == (O, I), (x.shape, weight.shape)
    x_flat = np.ascontiguousarray(x.reshape(TOK, I), dtype=np.float32)
    weight = np.ascontiguousarray(weight, dtype=np.float32)

    # per-tensor scales (two scalar reductions over the inputs)
    absmax = np.float32(np.abs(x_flat).max())
    wmean = np.float32(np.abs(weight).mean(dtype=np.float64))
    x_scale = np.float32(max(absmax, np.float32(EPS))) / np.float32(127.0)
    w_scale = np.float32(max(wmean, np.float32(EPS)))
    consts = np.zeros((1, 8), dtype=np.float32)
    consts[0, 0] = np.float32(1.0) / w_scale
    consts[0, 1] = np.float32(1.0) / x_scale
    consts[0, 2] = x_scale * w_scale

    # quantized matmul, tensor-parallel over out_features
    xT = np.ascontiguousarray(x_flat.T)               # [I, TOK]
    wTf = weight.T                                    # [I, O] view
    in_B = [{
        "xT": xT,
        "wT": np.ascontiguousarray(wTf[:, i * O_SH:(i + 1) * O_SH]),
        "consts": consts,
    } for i in range(N_CORES)]
    resB = _run(ncB, in_B, core_ids)
    out = np.concatenate([resB.results[i]["out"] for i in range(N_CORES)], axis=1)
    return out.reshape(B, T, O)
